# revision 20
# baseline (speedup 1.0000x reference)
"""Distributed causal self-attention kernel for Trainium2 (8 NeuronCores).

Problem: B=2, N=2048, D=1024, H=16 heads, Dh=64, fp32.
  q = x@Wq; k,v = x@Wkv; causal softmax(q k^T / sqrt(Dh)) @ v; out = .@Wo + bo

Sharding (8 cores): core c -> batch b = c//4, head group g = c%4 (4 heads).
Each core computes q/k/v projections and full causal attention for its 4
heads over the whole sequence locally in transposed [inner, seq] layout,
processing 512-query blocks (ic) in order. After both head-pairs of a query
block finish, the block's output is exchanged with one 8-core AllToAll:
core j receives the full inner dim (all 16 heads) for query rows
[512*ic + 64*j, +64) of BOTH batches and applies the full output projection
(complete Wo on every core) for those rows.

The emission schedule is paced by a coarse PE/ACT cost model: the per-block
exp on the scalar engine (~1 elem/cycle/partition) is the attention-phase
rate limiter, and any PE micro-idle re-throttles the tensor-engine HAM
clock gate to 1.2 GHz. Projection and output-projection matmuls are
therefore split into ~2-matmul micro-tasks and interleaved between
attention blocks exactly where the model predicts the PE would otherwise
wait on exp, keeping the PE stream back-to-back.

The softmax denominator rides the PV matmul as a 65th ones-column of v.
The divide uses a DVE reciprocal off the critical path (and Ln->Exp on the
scalar engine, ~4e-5 rel err, for the last block where latency matters),
then a PE ones-column broadcast matmul and one DVE multiply.
"""

import os
import sys
import types

import numpy as np
import ml_dtypes

BF16_NP = ml_dtypes.bfloat16

import concourse.bass as bass
import concourse.mybir as mybir
import concourse.tile as tile
from concourse.bass_utils import run_bass_kernel_spmd

F32 = mybir.dt.float32
BF16 = mybir.dt.bfloat16
AF = mybir.ActivationFunctionType
ALU = mybir.AluOpType

B, N, D = 2, 2048, 1024
H, DH = 16, 64
SCALE = DH ** -0.5
MASK_VAL = -30.0
KC = 8
GROUPS_A2A = [[0, 1, 2, 3, 4, 5, 6, 7]]

_counter = [0]


def _split_multi_waits(nc, limit=1):
    """This container's walrus accepts at most one sync wait per instruction;
    hoist extra waits onto standalone event-semaphore waits inserted just
    before the owning instruction in the same engine stream."""
    for bb in nc.main_func.blocks:
        insts = bb.instructions
        i = 0
        while i < len(insts):
            inst = insts[i]
            si = inst.sync_info
            if si is not None and len(si.on_wait) > limit:
                waits = list(si.on_wait)
                hoist, keep = waits[:-limit], waits[-limit:]
                for k, w in enumerate(hoist):
                    _counter[0] += 1
                    ies = mybir.InstEventSemaphore(
                        name=f"I-waitsplit-{_counter[0]}", ins=[], outs=[]
                    )
                    ies.engine = inst.engine
                    ies.sync_info = mybir.SyncInfo(on_wait=[w], on_update=[])
                    insts.insert(i + k, ies)
                inst.sync_info = mybir.SyncInfo(
                    on_wait=keep, on_update=list(si.on_update)
                )
                i += len(hoist)
            i += 1


def _install_prof_shim():
    """Let run_bass_kernel_spmd(trace=True)/BASS_TRACE work in this image:
    register the NTFF hook whose antenv.axon_hooks shim module is missing."""
    if "antenv.axon_hooks" in sys.modules:
        return
    try:
        mod = types.ModuleType("antenv.axon_hooks")
        _hook = [None]
        mod.set_axon_ntff_profile_hook = lambda h: _hook.__setitem__(0, h)
        mod.get_axon_ntff_profile_hook = lambda: _hook[0]
        sys.modules["antenv.axon_hooks"] = mod
        import antenv

        antenv.axon_hooks = mod
        from trn_agent_boot.trn_boot import _ntff_profile_via_ctypes

        mod.set_axon_ntff_profile_hook(
            _ntff_profile_via_ctypes("/opt/axon/libaxon_pjrt.so")
        )
    except Exception:
        pass


def _build():
    nc = bass.Bass("TRN2", target_bir_lowering=False, num_devices=8)

    xT_ext = nc.declare_dram_parameter("xT", [D, N], BF16, isOutput=False)
    wq_ext = nc.declare_dram_parameter("wq", [D, 256], BF16, isOutput=False)
    wk_ext = nc.declare_dram_parameter("wk", [D, 256], BF16, isOutput=False)
    wv_ext = nc.declare_dram_parameter("wv", [D, 256], BF16, isOutput=False)
    wo_ext = nc.declare_dram_parameter("wo", [D, D], BF16, isOutput=False)
    bo_ext = nc.declare_dram_parameter("bo", [1, D], BF16, isOutput=False)
    out_ext = nc.declare_dram_parameter("out", [4, 128, D], F32, isOutput=True)

    a2a_in = [nc.dram_tensor(f"a2a_in{ic}", [8, 2, 128, 64], BF16) for ic in range(4)]
    a2a_out = [nc.dram_tensor(f"a2a_out{ic}", [8, 2, 128, 64], BF16) for ic in range(4)]

    with tile.TileContext(nc) as tc, nc.allow_low_precision(
        reason="bf16 matmul tiles"
    ), (
        tc.tile_pool(name="sbA", bufs=1)
    ) as sbA, tc.tile_pool(name="sbP", bufs=4) as sbP, tc.tile_pool(
        name="sbS", bufs=2
    ) as sbS, tc.tile_pool(name="sbO", bufs=4) as sbO, tc.tile_pool(
        name="ps_s", bufs=2, space="PSUM"
    ) as ps_s, tc.tile_pool(name="ps_n", bufs=1, space="PSUM") as ps_n, tc.tile_pool(
        name="ps_w", bufs=2, space="PSUM"
    ) as ps_w:
        # ---- persistent tiles ----
        wo_sb = [sbA.tile([128, D], BF16, tag=f"wo{k}", name=f"wo{k}") for k in range(KC)]
        bo_sb = sbA.tile([1, D], BF16, tag="bo", name="bo")
        ones_row = sbA.tile([1, 128], BF16, tag="ones", name="ones")
        ones_col = sbA.tile([33, 64], BF16, tag="onesc", name="onesc")
        maskK = sbA.tile([128, 128], F32, tag="maskK", name="maskK")
        maskB = sbA.tile([128, 128], BF16, tag="maskB", name="maskB")
        identB = sbA.tile([128, 128], BF16, tag="identB", name="identB")
        identF = sbA.tile([128, 128], F32, tag="identF", name="identF")
        qT = [sbA.tile([128, N], BF16, tag=f"qT{p}", name=f"qT{p}") for p in range(2)]
        kT = [sbA.tile([128, N], BF16, tag=f"kT{p}", name=f"kT{p}") for p in range(2)]
        attnT = [sbA.tile([128, N], BF16, tag=f"attnT{p}", name=f"attnT{p}") for p in range(2)]
        # v layout: per j-tile block of 260 cols: 4x [64 data | 1 one]
        vv = sbA.tile([128, 16 * 260], BF16, tag="vv", name="vv")
        xT_sb = [sbA.tile([128, N], BF16, tag=f"xT{k}", name=f"xT{k}") for k in range(KC)]
        wq_sb = [sbA.tile([128, 256], BF16, tag=f"wq{k}", name=f"wq{k}") for k in range(KC)]
        wk_sb = [sbA.tile([128, 256], BF16, tag=f"wk{k}", name=f"wk{k}") for k in range(KC)]
        wv_sb = [sbA.tile([128, 256], BF16, tag=f"wv{k}", name=f"wv{k}") for k in range(KC)]

        # ---- initial DMA, ordered so the first projections unblock early ----
        for nt in range(4):
            cols = slice(512 * nt, 512 * (nt + 1))
            for k in range(KC):
                nc.sync.dma_start(xT_sb[k][:, cols], xT_ext[128 * k : 128 * (k + 1), cols])
                if nt == 0:
                    rows = slice(128 * k, 128 * (k + 1))
                    nc.sync.dma_start(wq_sb[k][:], wq_ext[rows, :])
                    nc.sync.dma_start(wk_sb[k][:], wk_ext[rows, :])
                    nc.sync.dma_start(wv_sb[k][:], wv_ext[rows, :])
        for k in range(KC):
            nc.sync.dma_start(wo_sb[k][:], wo_ext[128 * k : 128 * (k + 1), :])
        nc.sync.dma_start(bo_sb[:], bo_ext[:])

        # causal mask tile: keep 0 where col >= row, else MASK_VAL
        nc.gpsimd.memset(maskK[:], 0.0)
        nc.gpsimd.affine_select(
            out=maskK[:],
            in_=maskK[:],
            compare_op=ALU.is_ge,
            fill=MASK_VAL,
            base=0,
            pattern=[[1, 128]],
            channel_multiplier=-1,
        )
        nc.gpsimd.memset(identF[:], 0.0)
        nc.gpsimd.affine_select(
            out=identF[:],
            in_=identF[:],
            compare_op=ALU.not_equal,
            fill=1.0,
            base=0,
            pattern=[[-1, 128]],
            channel_multiplier=1,
        )
        nc.vector.tensor_copy(identB[:], identF[:])
        nc.vector.tensor_copy(maskB[:], maskK[:])
        nc.scalar.activation(ones_row[:], maskK[0:1, :], AF.Copy, bias=1.0, scale=0.0)
        nc.scalar.activation(ones_col[:], maskK[0:33, 0:64], AF.Copy, bias=1.0, scale=0.0)
        v_ones = vv[:].rearrange("r (jt hl c) -> r jt hl c", jt=16, hl=4)[:, :, :, 64:65]
        m_src = maskK[:].rearrange("r (a b c) -> r a b c", a=16, b=4)[:, :, :, 0:1]
        nc.scalar.activation(v_ones, m_src, AF.Copy, bias=1.0, scale=0.0)

        # ---- emission-time cost model (warm-clock estimates, ns) ----
        clk = {"pe": 0.0, "act": 0.0}

        def mm_cost(nfree):
            return nfree / 2.4 + 28.0

        # ---- chunked projection emitters (2 matmuls per micro-task) ----
        def qk_chunk(dst, wsb, mul, p, nt, i, hold):
            cols = slice(512 * nt, 512 * (nt + 1))
            if i == 0:
                hold["ps"] = ps_w.tile([128, 512], F32, tag="w", name=f"qk{p}_{nt}")
            ps = hold["ps"]
            for k in (2 * i, 2 * i + 1):
                nc.tensor.matmul(
                    ps[:],
                    wsb[k][:, 128 * p : 128 * (p + 1)],
                    xT_sb[k][:, cols],
                    start=(k == 0),
                    stop=(k == KC - 1),
                )
            clk["pe"] += 2 * mm_cost(512)
            if i == 3:
                if mul is None:
                    nc.vector.tensor_copy(dst[p][:, cols], ps[:])
                else:
                    nc.vector.tensor_scalar_mul(dst[p][:, cols], ps[:], mul)

        def v_chunk(jt, i, hold):
            if i == 0:
                hold["ps"] = ps_w.tile([128, 512], F32, tag="w", name=f"vps{jt}")
            ps = hold["ps"]
            for k in (2 * i, 2 * i + 1):
                nc.tensor.matmul(
                    ps[:, 0:256],
                    xT_sb[k][:, 128 * jt : 128 * (jt + 1)],
                    wv_sb[k][:],
                    start=(k == 0),
                    stop=(k == KC - 1),
                )
            clk["pe"] += 2 * mm_cost(256)
            if i == 3:
                for hl in range(4):
                    nc.vector.tensor_copy(
                        vv[:, 260 * jt + 65 * hl : 260 * jt + 65 * hl + 64],
                        ps[:, 64 * hl : 64 * (hl + 1)],
                    )

        lhsTs = {}

        def out_chunk(ic, dh, i, hold):
            dcol = slice(512 * dh, 512 * (dh + 1))
            if i == 0:
                hold["ps"] = ps_w.tile([128, 512], F32, tag="w", name=f"ops{ic}_{dh}")
                nc.tensor.matmul(
                    hold["ps"][:], ones_row[:], bo_sb[0:1, dcol], start=True, stop=False
                )
                clk["pe"] += mm_cost(512)
            ps = hold["ps"]
            for k in (2 * i, 2 * i + 1):
                nc.tensor.matmul(
                    ps[:],
                    lhsTs[ic][k][:],
                    wo_sb[k][:, dcol],
                    start=False,
                    stop=(k == KC - 1),
                )
            clk["pe"] += 2 * mm_cost(512)
            if i == 3:
                osb = sbO.tile([128, 512], F32, tag="osb", name="osb", bufs=4)
                nc.vector.tensor_copy(osb[:], ps[:])
                nc.sync.dma_start(out_ext[ic, :, dcol], osb[:])

        # ---- filler queue: list of dicts(fn, nb=not-before-pe-clock, dl) ----
        # dl=(ic,bi) is a correctness deadline: the task writes data a block
        # at/after (ic,bi) reads, so it must be emitted before that block.
        fillq = []
        open_rest = []  # remaining chunks of a partially-emitted fill task

        def add_emit(chunk_fn, nargs, nb=0.0, dl=None):
            hold = {}
            group = []
            for i in range(4):
                t = {
                    "fn": (lambda i=i, hold=hold: chunk_fn(*nargs, i, hold)),
                    "nb": nb,
                    "dl": dl,
                    "grp": group,
                    "i": i,
                }
                group.append(t)
                fillq.append(t)

        def run_task(t):
            fillq.remove(t)
            if t["i"] == 0:
                open_rest.clear()
                open_rest.extend(t["grp"][1:])
            else:
                open_rest.remove(t)
            t["fn"]()

        def drain_open():
            # a chunked fill shares the ps_w ring with evac's broadcast
            # tiles; never leave one half-open across an evac
            while open_rest:
                run_task(open_rest[0])

        def pop_due(key):
            for t in [t for t in fillq if t["dl"] is not None and t["dl"] <= key]:
                run_task(t)

        def pop_until(target_pe):
            while clk["pe"] < target_pe:
                t = next((t for t in fillq if t["nb"] <= clk["pe"]), None)
                if t is None:
                    break
                run_task(t)

        # ---- attention blocks ----
        numTs = {}
        gateN = {0: 0.0, 1: 0.0}

        def scores_of(p, ic, jt):
            t = jt - 4 * ic
            lo = 128 * t if t >= 0 else 0
            jcol = slice(128 * jt, 128 * (jt + 1))
            sp = ps_s.tile([128, 1024], F32, tag="s", name="s_ps")
            for e in range(2):
                if t >= 0:
                    nc.tensor.matmul(
                        sp[:, 512 * e + lo : 512 * e + lo + 128],
                        identB[:],
                        maskB[:],
                        start=True,
                        stop=False,
                    )
                nc.tensor.matmul(
                    sp[:, 512 * e + lo : 512 * (e + 1)],
                    kT[p][64 * e : 64 * (e + 1), jcol],
                    qT[p][64 * e : 64 * (e + 1), 512 * ic + lo : 512 * (ic + 1)],
                    start=(t < 0),
                    stop=True,
                    tile_position=(64 * e, 0),
                )
            nfree = 512 - lo
            clk["pe"] += mm_cost(nfree) + (2 * mm_cost(128) if t >= 0 else 0.0)
            pT = sbP.tile([128, 1024], BF16, tag="pT", name="pT")
            sp3 = sp[:].rearrange("r (e w) -> r e w", e=2)[:, :, lo:512]
            pT3 = pT[:].rearrange("r (e w) -> r e w", e=2)[:, :, lo:512]
            nc.scalar.activation(pT3, sp3, AF.Exp)
            exp_done = max(clk["act"], clk["pe"]) + (2 * nfree + 352) / 1.2
            clk["act"] = exp_done
            return pT, lo, exp_done

        def pv_of(p, ic, jt, pT, lo, exp_done):
            njt = 4 * ic + 4
            if jt == 0:
                # numT banks are recycled across (p, ic) generations; make
                # sure the PE has other work until the previous divide frees
                # them, then until this block's exp lands
                pop_until(gateN[p])
                numTs[p] = ps_n.tile([65, 1024], F32, tag="n", name=f"num{p}_{ic}")
            pop_until(exp_done + 60.0)
            for e in range(2):
                vcol = 260 * jt + 65 * (2 * p + e)
                nc.tensor.matmul(
                    numTs[p][:, 512 * e + lo : 512 * (e + 1)],
                    vv[:, vcol : vcol + 65],
                    pT[:, 512 * e + lo : 512 * (e + 1)],
                    start=(jt == 0),
                    stop=(jt == njt - 1),
                )
            clk["pe"] += 2 * mm_cost(512 - lo)
            if jt == njt - 1:
                evac(p, ic)

        def evac(p, ic):
            drain_open()
            icol = slice(512 * ic, 512 * (ic + 1))
            numT = numTs[p]
            last = p == 1 and ic == 3
            if last:
                # latency-critical: 1/den via Ln->Exp on the scalar engine
                # (measured max rel err ~4e-5)
                lnd = sbS.tile([1, 1024], F32, tag="lnd", name="lnd", bufs=2)
                recip_t = sbS.tile([1, 1024], BF16, tag="lrec", name="lrec", bufs=2)
                nc.scalar.activation(lnd[:], numT[64:65, :], AF.Ln)
                nc.scalar.activation(recip_t[:], lnd[:], AF.Exp, scale=-1.0)
                rslice = lambda e: recip_t[0:1, 512 * e : 512 * (e + 1)]
                oslice = lambda e: ones_col[0:1, :]
                chain = 2600.0
            else:
                den_t = sbS.tile([33, 512], F32, tag="dent", name="dent", bufs=2)
                for e in range(2):
                    nc.vector.tensor_copy(
                        den_t[32 * e : 32 * e + 1, :],
                        numT[64:65, 512 * e : 512 * (e + 1)],
                    )
                recip_t = sbS.tile([33, 512], BF16, tag="recr", name="recr", bufs=2)
                nc.vector.reciprocal(recip_t[:], den_t[:])
                rslice = lambda e: recip_t[32 * e : 32 * e + 1, :]
                oslice = lambda e: ones_col[32 * e : 32 * e + 1, :]
                chain = 5200.0
            for e in range(2):
                ecol = slice(512 * e, 512 * (e + 1))
                numsb = sbS.tile([64, 512], F32, tag="numsb", name="numsb", bufs=4)
                nc.vector.tensor_copy(numsb[:], numT[0:64, ecol])
                rb = ps_w.tile([128, 512], F32, tag="w", name="rb")
                nc.tensor.matmul(
                    rb[0:64, :], oslice(e), rslice(e), start=True, stop=True
                )
                clk["pe"] += mm_cost(512)
                nc.vector.tensor_tensor(
                    attnT[p][64 * e : 64 * (e + 1), icol],
                    numsb[:],
                    rb[0:64, :],
                    op=ALU.mult,
                )
            gateN[p] = clk["pe"] + chain
            for j in range(8):
                nc.sync.dma_start(
                    a2a_in[ic][j, p],
                    attnT[p][:, 512 * ic + 64 * j : 512 * ic + 64 * j + 64],
                )
            if p == 1:
                nc.gpsimd.collective_compute(
                    "AllToAll",
                    ALU.bypass,
                    ins=[a2a_in[ic][:]],
                    outs=[a2a_out[ic][:]],
                    replica_groups=GROUPS_A2A,
                )
                tiles = {}
                for k in range(KC):
                    lh = sbO.tile(
                        [128, 128], BF16, tag=f"lh{k}", name=f"lh{k}_{ic}", bufs=2
                    )
                    nc.sync.dma_start(lh[:, 0:64], a2a_out[ic][k // 2, k % 2])
                    nc.sync.dma_start(lh[:, 64:128], a2a_out[ic][4 + k // 2, k % 2])
                    tiles[k] = lh
                lhsTs[ic] = tiles

        # ---- prereq projections for ic=0 (no exp pressure yet) ----
        hold = {}
        for i in range(4):
            qk_chunk(qT, wq_sb, SCALE, 0, 0, i, hold)
        hold = {}
        for i in range(4):
            qk_chunk(kT, wk_sb, None, 0, 0, i, hold)
        hold = {}
        for i in range(4):
            qk_chunk(qT, wq_sb, SCALE, 1, 0, i, hold)
        hold = {}
        for i in range(4):
            qk_chunk(kT, wk_sb, None, 1, 0, i, hold)
        for jt in range(4):
            hold = {}
            for i in range(4):
                v_chunk(jt, i, hold)

        # ---- filler queue: qk(nt) due at ic=nt start; v(jt) due right
        # before its first consuming pv; out(ic) gated behind its A2A ----
        for nt in (1, 2, 3):
            for p in range(2):
                add_emit(qk_chunk, (qT, wq_sb, SCALE, p, nt), dl=(nt, 0))
                add_emit(qk_chunk, (kT, wk_sb, None, p, nt), dl=(nt, 0))
        for jt in range(4, 16):
            add_emit(v_chunk, (jt,), dl=(jt // 4, jt))
        a2a_gate = {}

        # ---- main ic-major loop ----
        for ic in range(4):
            if 1 <= ic <= 2:
                for dh in range(2):
                    add_emit(out_chunk, (ic - 1, dh), nb=a2a_gate[ic - 1])
            blocks = [(p, jt) for p in range(2) for jt in range(4 * ic + 4)]
            pend = None
            for bi, (p, jt) in enumerate(blocks):
                pop_due((ic, bi))
                cur = scores_of(p, ic, jt)
                if pend is not None:
                    pv_of(*pend)
                pend = (p, ic, jt) + cur
            pv_of(*pend)  # trailing pv triggers evac(1, ic) -> A2A(ic)
            a2a_gate[ic] = clk["pe"] + 11000.0
        # drain remaining fillers (out(0..2) leftovers), then the final
        # out-projections; out(3)'s chunks ride out the last collective
        while fillq:
            t = fillq.pop(0)
            t["fn"]()
        for oc in (2, 3):
            for dh in range(2):
                hold = {}
                for i in range(4):
                    out_chunk(oc, dh, i, hold)

    _split_multi_waits(nc)
    return nc


_NC_CACHE = {}


def _get_nc():
    if "nc" not in _NC_CACHE:
        _NC_CACHE["nc"] = _build()
    return _NC_CACHE["nc"]


def kernel(x, Wq, Wkv, Wo, bo):
    _install_prof_shim()
    x = np.ascontiguousarray(np.asarray(x, dtype=np.float32))
    Wq = np.ascontiguousarray(np.asarray(Wq, dtype=np.float32))
    Wkv = np.ascontiguousarray(np.asarray(Wkv, dtype=np.float32))
    Wo = np.ascontiguousarray(np.asarray(Wo, dtype=np.float32))
    bo = np.ascontiguousarray(np.asarray(bo, dtype=np.float32))

    xT = [np.ascontiguousarray(x[b].T).astype(BF16_NP) for b in range(B)]
    wo_bf = np.ascontiguousarray(Wo).astype(BF16_NP)
    bo_bf = np.ascontiguousarray(bo[None, :]).astype(BF16_NP)
    in_maps = []
    for c in range(8):
        b, g = divmod(c, 4)
        cols = slice(256 * g, 256 * (g + 1))
        in_maps.append(
            {
                "xT": xT[b],
                "wq": np.ascontiguousarray(Wq[:, cols]).astype(BF16_NP),
                "wk": np.ascontiguousarray(Wkv[:, cols]).astype(BF16_NP),
                "wv": np.ascontiguousarray(Wkv[:, 1024:][:, cols]).astype(BF16_NP),
                "wo": wo_bf,
                "bo": bo_bf,
            }
        )

    nc = _get_nc()
    trace = bool(int(os.environ.get("KERNEL_TRACE", "0")))
    import time as _time

    last_exc = None
    for attempt in range(3):
        try:
            res = run_bass_kernel_spmd(
                nc, in_maps, core_ids=list(range(8)), trace=trace
            )
            break
        except Exception as exc:  # noqa: BLE001
            last_exc = exc
            _time.sleep(5.0)
    else:
        raise last_exc
    if trace:
        kernel.last_exec_time_ns = res.exec_time_ns

    # core j returns [4 ic, 128, 1024]: rows 0:64 = batch 0 rows
    # [512*ic + 64*j, +64), rows 64:128 = same rows of batch 1
    out = np.empty((B, N, D), dtype=np.float32)
    for j in range(8):
        r = res.results[j]["out"]
        for ic in range(4):
            rows = slice(512 * ic + 64 * j, 512 * ic + 64 * j + 64)
            out[0, rows, :] = r[ic, 0:64, :]
            out[1, rows, :] = r[ic, 64:128, :]
    return out
